# revision 1
# baseline (speedup 1.0000x reference)
"""Trainium2 Bass kernel for nn_Attention_9594956939856 (v3).

Single-head spatial self-attention over 64x64 feature maps:
    q = Wq@x + bq, k = Wk@x + bk, v = Wv@x + bv   (1x1 convs)
    out = gamma * softmax(q^T k) @ v + x

Sharding: data-parallel over batch - 8 samples onto 8 NeuronCores; no
collectives. Per core: C=256 channels, N=4096 tokens, dk=32.

Design (engine costs MEASURED on this hardware via micro.py, which
disagrees badly with the shipped cost model):
  - bf16 matmuls stream 2 cols/cycle here (~109ns per (128,512) out), fp8
    DoubleRow ~142ns, but K=32 matmuls stream 4-6x slower (SBUF feeds one
    value per partition per cycle, so only 32 of 128 rows supply data).
    The scores therefore contract ALL 128 partitions of the 4x-replicated
    q/k (the 4x overcount is folded into Wq/bq host-side), computing the
    TRANSPOSED scores s'[j,i] so the attention-weighted sum needs no
    transpose of the huge matrix.
  - ACT exp is very fast here (227ns per (128,1024) PSUM->fp8), so ALL
    exp runs on ACT. ACT Copy/Identity are SLOW (0.8-1.6us) - ACT does
    exp ONLY.
  - accumulation: fp8 DoubleRow over j-pairs: po0/po1 (2x128 channels,
    vT stationary) and the denominator pd via an all-ones lhsT with
    M=128, which lands d[i] broadcast across all 128 PSUM partitions
    (no quadrant-sum / broadcast matmul needed).
  - the accumulation matmuls are software-pipelined LAG pairs behind the
    scores matmuls, so the in-order PE never sits in a pure-scores phase
    stalled on exp slot-reuse (the v1 structure serialized here).
  - PSUM->SBUF casts (q/k bias-add, vt8) and the finalize chain on DVE
    (the PE stream is the floor; DVE sits at ~25% load); gamma*bv
    residual term folded into x host-side (exact).
"""

import numpy as np
import ml_dtypes

import concourse.bass as bass
import concourse.mybir as mybir
from concourse.tile import TileContext
from concourse.bass_utils import run_bass_kernel_spmd

B, C, H, W = 8, 256, 64, 64
N = H * W          # 4096 tokens
DK = C // 8        # 32
P = 128
F32 = mybir.dt.float32
BF16 = mybir.dt.bfloat16
FP8 = mybir.dt.float8e4
DR = mybir.MatmulPerfMode.DoubleRow
AF = mybir.ActivationFunctionType
ALU = mybir.AluOpType

NJT = N // P       # 32 j-tiles
NJP = NJT // 2     # 16 j-pairs
HCH = 512          # i-chunk width
NCH = N // HCH     # 8 i-chunks

VARIANT = {
    "acc_lag": 3,        # j-pairs of lookahead before accumulation
    "xb_on_pool": False,  # xb casts on gpsimd instead of DVE
}


# ---------------------------------------------------------------------------
# Workaround: the walrus build in this container allows only ONE sync wait
# per instruction ("Too many sync wait commands"), but Tile's wait
# assignment attaches up to 2 (and the tail drain more). Hoist all-but-one
# wait of any over-subscribed instruction onto dedicated same-engine nofuse
# nops inserted immediately before it in the ordered stream.
_PATCHED = False


def _apply_tile_patch():
    global _PATCHED
    if _PATCHED:
        return
    from concourse.tile import TileContext as TC
    from concourse.vector_clock import ScopedClock, VectorClock

    def _drain_and_barrier_split(self, tick_clock, wait_clock):
        gc = tick_clock.global_clock
        n = len(gc)
        for i in range(n):
            if gc[i] > 0:
                vec = [0] * n
                vec[i] = gc[i]
                ins = self.nc.sync.nop(nofuse=True, hint="tail_drain_wait")
                wait_clock.add_sem_waits(
                    ins.ins, ScopedClock({None: VectorClock(vec)})
                )
        self.nc.sync.drain()
        self.nc.all_engine_barrier()
        assert self.sems is not None
        popped = self.nc._tile_sem_poison_stack.pop()
        assert popped is self._sem_poison
        self.nc.clear_and_free_semaphores(list(self.sems.allocated().values()))
        self.nc.all_engine_barrier()

    TC._drain_and_barrier = _drain_and_barrier_split

    orig_lower = TC._lower_ordered_insts
    counter = [0]

    def _lower_split_waits(self, ordered):
        for bb_name, insts in ordered.items():
            new = []
            changed = False
            for inst in insts:
                si = inst.sync_info
                if si is not None and len(si.on_wait) > 1:
                    # coalesce waits on the same semaphore to the max
                    # target first (all waits are sem-ge-imm, so this is
                    # semantics-preserving) -- avoids the nop split
                    waits = list(si.on_wait)
                    mergeable = all(
                        w.sync_type == "semaphore"
                        and w.wait_mode == "sem-ge-imm"
                        and w.wait_reg is None
                        for w in waits
                    )
                    if mergeable:
                        best, order = {}, []
                        for w in waits:
                            if w.id in best:
                                if w.wait_value > best[w.id].wait_value:
                                    best[w.id] = w
                            else:
                                best[w.id] = w
                                order.append(w.id)
                        waits = [best[k] for k in order]
                    if len(waits) == 1:
                        inst.sync_info = mybir.SyncInfo(
                            on_wait=waits, on_update=list(si.on_update)
                        )
                        new.append(inst)
                        continue
                    changed = True
                    for w in waits[:-1]:
                        counter[0] += 1
                        new.append(
                            mybir.InstNoOp(
                                name=f"splitw-{counter[0]}",
                                sync_info=mybir.SyncInfo(
                                    on_wait=[w], on_update=[]
                                ),
                                bass_nofuse=True,
                                engine=inst.engine,
                            )
                        )
                    inst.sync_info = mybir.SyncInfo(
                        on_wait=[waits[-1]], on_update=list(si.on_update)
                    )
                new.append(inst)
            if changed:
                insts[:] = new
        return orig_lower(self, ordered)

    TC._lower_ordered_insts = _lower_split_waits
    _PATCHED = True


def _emit_body(nc, tc, pools, ext):
    consts, big, epool, fin, ps_s_pool, ps_acc_pool = pools
    x_e, wqt_e, wkt_e, wvt_e, bq_e, bk_e, gam_e, y_e = ext

    # ---- constants / weights ---------------------------------------------
    wqt_a = consts.tile([P, P], BF16, tag="wqt_a")
    wqt_b = consts.tile([P, P], BF16, tag="wqt_b")
    wkt_a = consts.tile([P, P], BF16, tag="wkt_a")
    wkt_b = consts.tile([P, P], BF16, tag="wkt_b")
    wvt_a = consts.tile([P, C], BF16, tag="wvt_a")
    wvt_b = consts.tile([P, C], BF16, tag="wvt_b")
    bq_t = consts.tile([P, 1], F32, tag="bq_t")
    bk_t = consts.tile([P, 1], F32, tag="bk_t")
    gam_t = consts.tile([P, 1], F32, tag="gam_t")
    ones8 = consts.tile([P, 2 * P], FP8, tag="ones8")

    nc.sync.dma_start(out=wqt_a[:], in_=wqt_e[0:P, :])
    nc.sync.dma_start(out=wqt_b[:], in_=wqt_e[P : 2 * P, :])
    nc.sync.dma_start(out=wkt_a[:], in_=wkt_e[0:P, :])
    nc.sync.dma_start(out=wkt_b[:], in_=wkt_e[P : 2 * P, :])
    nc.sync.dma_start(out=wvt_a[:], in_=wvt_e[0:P, :])
    nc.sync.dma_start(out=wvt_b[:], in_=wvt_e[P : 2 * P, :])
    nc.sync.dma_start(out=bq_t[:], in_=bq_e[:])
    nc.sync.dma_start(out=bk_t[:], in_=bk_e[:])
    nc.sync.dma_start(out=gam_t[:], in_=gam_e[:])
    nc.vector.memset(ones8[:], 1.0)
    ones_ap = ones8[:].rearrange("p (o m) -> p o m", o=2)

    # ---- big SBUF tensors -------------------------------------------------
    xf0 = big.tile([P, N], F32, tag="xf0")     # channels 0..127 (+g*bv)
    xf1 = big.tile([P, N], F32, tag="xf1")
    xb0 = big.tile([P, N], BF16, tag="xb0")
    xb1 = big.tile([P, N], BF16, tag="xb1")
    q_rep = big.tile([P, N], BF16, tag="q_rep")
    k_rep = big.tile([P, N], BF16, tag="k_rep")
    # vt8 pair layout: block jp holds cols jp*512 + h*256 + o*128 + m ==
    # vT[j=(2jp+o)*128+p, channel h*128+m]
    vt8 = big.tile([P, NJP * 512], FP8, tag="vt8")

    # ---- prologue: x load + bf16 casts + projections ---------------------
    for nch in range(NCH):
        sl = slice(nch * HCH, (nch + 1) * HCH)
        nc.sync.dma_start(out=xf0[:, sl], in_=x_e[0:P, sl])
        nc.sync.dma_start(out=xf1[:, sl], in_=x_e[P : 2 * P, sl])
        nc.vector.tensor_copy(xb0[:, sl], xf0[:, sl])
        nc.vector.tensor_copy(xb1[:, sl], xf1[:, sl])

    # k/q projections: 1024-col PSUM tiles (2 i-chunks), one biased cast
    for proj_w, proj_bias, proj_out in (
        (("wk", wkt_a, wkt_b), bk_t, k_rep),
        (("wq", wqt_a, wqt_b), bq_t, q_rep),
    ):
        _, w_a, w_b = proj_w
        for half in range(NCH // 2):
            pk = ps_s_pool.tile(
                [P, 2 * HCH], F32, tag="ps", bufs=2, name=f"pk{half}"
            )
            for o in range(2):
                sl = slice((2 * half + o) * HCH, (2 * half + o + 1) * HCH)
                psl = slice(o * HCH, (o + 1) * HCH)
                nc.tensor.matmul(
                    pk[:, psl], w_a, xb0[:, sl], start=True, stop=False
                )
                nc.tensor.matmul(
                    pk[:, psl], w_b, xb1[:, sl], start=False, stop=True
                )
            osl = slice(half * 2 * HCH, (half + 1) * 2 * HCH)
            nc.vector.tensor_scalar_add(proj_out[:, osl], pk[:], proj_bias[:])

    # v projection: 4 j-tiles per PSUM tile at natural (jloc, h, m) order,
    # one rearranging fp8 cast into the vt8 pair layout per group
    for grp in range(NJT // 4):
        pv = ps_s_pool.tile([P, 2 * HCH], F32, tag="ps", bufs=2)
        for jloc in range(4):
            jt = 4 * grp + jloc
            nsl = slice(jt * P, (jt + 1) * P)
            psl = slice(jloc * C, (jloc + 1) * C)
            nc.tensor.matmul(
                pv[:, psl], xb0[:, nsl], wvt_a, start=True, stop=False
            )
            nc.tensor.matmul(
                pv[:, psl], xb1[:, nsl], wvt_b, start=False, stop=True
            )
        # read (pr, o, h, m) [strides 512,256,128,1]; write vt8 (pr, h, o, m)
        src = pv[:].rearrange("p (r o h m) -> p r o h m", r=2, o=2, h=2)
        dst = vt8[
            :, grp * 1024 : (grp + 1) * 1024
        ].rearrange("p (r h o m) -> p r o h m", r=2, h=2, o=2)
        nc.vector.tensor_copy(dst, src)

    # ---- main attention loop over i-chunks -------------------------------
    # Pair jp's scores+exp emit together with pair jp-LAG's accumulation,
    # keeping the in-order PE busy while ACT drains the exp queue.
    LAG = VARIANT["acc_lag"]
    for ich in range(NCH):
        isl = slice(ich * HCH, (ich + 1) * HCH)
        es = []

        po0 = ps_acc_pool.tile([P, HCH], F32, tag="po", bufs=2)
        po1 = ps_acc_pool.tile([P, HCH], F32, tag="po", bufs=2)
        pd = ps_acc_pool.tile([P, HCH], F32, tag="pd", bufs=2)

        def _accum(jp):
            rhs = es[jp][:].rearrange("p (o i) -> p o i", o=2)
            st, sp = jp == 0, jp == NJP - 1
            nc.tensor.matmul(
                pd[:], ones_ap, rhs, start=st, stop=sp, perf_mode=DR
            )
            for h, po in ((0, po0), (1, po1)):
                lhsT = vt8[
                    :, jp * 512 + h * 2 * P : jp * 512 + (h + 1) * 2 * P
                ].rearrange("p (o m) -> p o m", o=2)
                nc.tensor.matmul(
                    po[:], lhsT, rhs, start=st, stop=sp, perf_mode=DR
                )

        for jp in range(NJP + LAG):
            if jp < NJP:
                ps = ps_s_pool.tile([P, 2 * HCH], F32, tag="ps", bufs=2)
                for o in range(2):
                    jt = 2 * jp + o
                    nc.tensor.matmul(
                        ps[:, o * HCH : (o + 1) * HCH],
                        k_rep[:, jt * P : (jt + 1) * P],
                        q_rep[:, isl],
                        start=True, stop=True,
                    )
                e8 = epool.tile([P, 2 * HCH], FP8, tag="e", bufs=24)
                nc.scalar.activation(e8[:], ps[:], AF.Exp)
                es.append(e8)
            if jp >= LAG:
                _accum(jp - LAG)

        # finalize: out = po * (gamma / d) + x
        dr = fin.tile([P, HCH], F32, tag="dr", bufs=2)
        nc.vector.reciprocal(dr[:], pd[:])
        nc.vector.tensor_scalar_mul(dr[:], dr[:], gam_t[:])
        m0 = fin.tile([P, HCH], F32, tag="m0", bufs=2)
        t0 = fin.tile([P, HCH], F32, tag="t0", bufs=2)
        nc.vector.tensor_tensor(m0[:], po0[:], dr[:], op=ALU.mult)
        nc.vector.tensor_tensor(t0[:], m0[:], xf0[:, isl], op=ALU.add)
        # y-DMA triggers on the SP queue (idle during the main loop): on the
        # in-order ACT queue they wait for the DVE finalize and head-of-line
        # block the next chunk's exp instructions, stalling the scores
        # pipeline through the ps-slot WAR at every chunk boundary
        nc.sync.dma_start(out=y_e[0:P, isl], in_=t0[:])
        m1 = fin.tile([P, HCH], F32, tag="m1", bufs=2)
        t1 = fin.tile([P, HCH], F32, tag="t1", bufs=2)
        nc.vector.tensor_tensor(m1[:], po1[:], dr[:], op=ALU.mult)
        nc.vector.tensor_tensor(t1[:], m1[:], xf1[:, isl], op=ALU.add)
        nc.sync.dma_start(out=y_e[P : 2 * P, isl], in_=t1[:])


def build_bass(loop_n: int | None = None) -> bass.Bass:
    """Build the kernel. loop_n wraps the body in a device-side For_i loop
    (with a tiny 'tick' sentinel output) for slope-based benchmarking."""
    _apply_tile_patch()
    nc = bass.Bass()

    x_e = nc.declare_dram_parameter("x", [C, N], F32, isOutput=False)
    wqt_e = nc.declare_dram_parameter("wqt", [C, P], BF16, isOutput=False)
    wkt_e = nc.declare_dram_parameter("wkt", [C, P], BF16, isOutput=False)
    wvt_e = nc.declare_dram_parameter("wvt", [C, C], BF16, isOutput=False)
    bq_e = nc.declare_dram_parameter("bq_r", [P, 1], F32, isOutput=False)
    bk_e = nc.declare_dram_parameter("bk_r", [P, 1], F32, isOutput=False)
    gam_e = nc.declare_dram_parameter("gam_b", [P, 1], F32, isOutput=False)
    y_e = nc.declare_dram_parameter("y", [C, N], F32, isOutput=True)
    tick_e = None
    if loop_n is not None:
        tick_e = nc.declare_dram_parameter("tick", [1, 8], F32, isOutput=True)

    ext = (x_e, wqt_e, wkt_e, wvt_e, bq_e, bk_e, gam_e, y_e)

    with (
        TileContext(nc) as tc,
        tc.tile_pool(name="consts", bufs=1) as consts,
        tc.tile_pool(name="big", bufs=2) as big,
        tc.tile_pool(name="epool", bufs=24) as epool,
        tc.tile_pool(name="fin", bufs=2) as fin,
        tc.tile_pool(name="ps_s", bufs=2, space="PSUM") as ps_s_pool,
        tc.tile_pool(name="ps_acc", bufs=2, space="PSUM") as ps_acc_pool,
    ):
        pools = (consts, big, epool, fin, ps_s_pool, ps_acc_pool)
        if loop_n is None:
            _emit_body(nc, tc, pools, ext)
        else:
            with tc.For_i(0, loop_n, 1):
                _emit_body(nc, tc, pools, ext)
            t = fin.tile([1, 8], F32, tag="tick")
            nc.vector.memset(t[:], 1.0)
            nc.sync.dma_start(out=tick_e[:], in_=t[:])

    return nc


_NC_CACHE = None


def _get_nc() -> bass.Bass:
    global _NC_CACHE
    if _NC_CACHE is None:
        _NC_CACHE = build_bass()
    return _NC_CACHE


def prep_core_inputs(x, Wq, bq, Wk, bk, Wv, bv, gamma):
    x = np.asarray(x, np.float32).reshape(B, C, N)
    Wq = np.asarray(Wq, np.float32)
    Wk = np.asarray(Wk, np.float32)
    Wv = np.asarray(Wv, np.float32)
    bq = np.asarray(bq, np.float32)
    bk = np.asarray(bk, np.float32)
    bv = np.asarray(bv, np.float32)
    g = float(np.asarray(gamma, np.float32).reshape(-1)[0])

    # residual absorbs gamma*bv (exact): out = gamma*(v_hat@attn) + (x+g*bv)
    xadj = x + (g * bv)[None, :, None]

    # scores contract all 128 partitions = 4 replicas of the 32 q-dims;
    # the 4x overcount is folded into Wq (and bq) here
    wqt = np.ascontiguousarray(np.tile(0.25 * Wq.T, (1, 4))).astype(
        ml_dtypes.bfloat16
    )
    wkt = np.ascontiguousarray(np.tile(Wk.T, (1, 4))).astype(
        ml_dtypes.bfloat16
    )
    wvt = np.ascontiguousarray(Wv.T).astype(ml_dtypes.bfloat16)
    bq_r = np.ascontiguousarray(np.tile(0.25 * bq, 4)).reshape(P, 1)
    bk_r = np.ascontiguousarray(np.tile(bk, 4)).reshape(P, 1)
    gam_b = np.full((P, 1), g, np.float32)

    shared = {
        "wqt": wqt, "wkt": wkt, "wvt": wvt,
        "bq_r": bq_r, "bk_r": bk_r, "gam_b": gam_b,
    }
    return [
        {"x": np.ascontiguousarray(xadj[b]), **shared} for b in range(B)
    ]


def kernel(**inputs) -> np.ndarray:
    nc = _get_nc()
    in_maps = prep_core_inputs(**inputs)
    res = run_bass_kernel_spmd(nc, in_maps, list(range(B)))
    y = np.stack([res.results[i]["y"] for i in range(B)])
    return np.ascontiguousarray(y.reshape(B, C, H, W).astype(np.float32))

